# revision 1
# baseline (speedup 1.0000x reference)
"""Trainium2 Bass kernel: Bahdanau-style attention scores + softmax.

Reference computation (all fp32 in the oracle):
    Wh = attn_w[:, :H]; We = attn_w[:, H:]            # [K, H] each (K == H == 512)
    h_proj[b, k] = sum_h hidden[b, h] * Wh[k, h] + attn_b[k]
    e_proj[b, s, k] = sum_h enc[b, s, h] * We[k, h]
    scores[b, s] = sum_k v[k] * tanh(h_proj[b, k] + e_proj[b, s, k])
    out = softmax(scores, axis=s)

Strategy: pure data-parallel over batch (32 -> 4 per core, 8 cores). No
collectives needed (softmax axis lives entirely on one core).

Per-core device layout (k on partitions — "layout A"):
    e_projT[k, s] = sum_h WeT[h, k] * encT[h, s]
    - enc is staged HOST-SIDE transposed to [H, S] and cast to bf16, so the
      contraction dim h lands on partitions with a perfectly contiguous DMA
      and zero on-device transposes (PE does only the real matmuls).
    - h_proj is computed on-device (tiny matmul) and folded into the tanh as
      the ACT engine's per-partition bias: energyT = tanh(e_projT + h_projT).
    - scores = v . energyT via PE matvec with v replicated to M=128 columns, so
      the scores row is replicated across all partitions and every softmax op
      runs on full base-partition-0 tiles (other base partitions / exotic
      reduce flavors crashed at NRT execute time on this toolchain). Measured
      alternatives that LOST on HW: M=1 stationary (+11us/iter), v-dot via
      DVE + gpsimd.partition_all_reduce (+132us/iter — Tile serializes GPSIMD
      against concurrent DVE).
    - softmax is flash-style: per 512-column chunk, DVE computes the chunk max
      off the score PSUM tile and ACT computes exp(s - max_c) with an
      accumulated chunk sum; a small per-batch epilogue merges the chunk
      normalizers and rescales, overlapping everything but the last batch's
      epilogue with the matmul stream.
"""

import os
import sys

import numpy as np

for _p in ("/opt/trn_rl_repo", "/root/.axon_site/_ro/trn_rl_repo"):
    if os.path.isdir(_p) and _p not in sys.path:
        sys.path.insert(0, _p)

import ml_dtypes

B, S, H = 32, 4096, 512
NCORES = 8
BL = B // NCORES          # batches per core
P = 128                   # partitions
KB = H // P               # k blocks (output dim of the projection)
HB = H // P               # h blocks (contraction dim)
CH = 512                  # seq columns per psum tile
BF16 = ml_dtypes.bfloat16


def build_nc(bl=BL, s=S, reps=1):
    """Build the per-core Bass program.

    reps>1 wraps the main computation in a hardware For_i loop repeating the
    identical work — used only for wall-clock benchmarking (device time scales
    with reps while the fixed axon RPC overhead does not).
    """
    import concourse.bass as bass  # noqa: F401
    import concourse.mybir as mybir
    import concourse.tile as tile
    from concourse import bacc
    from contextlib import ExitStack, nullcontext

    f32 = mybir.dt.float32
    b16 = mybir.dt.bfloat16
    Tanh = mybir.ActivationFunctionType.Tanh
    Exp = mybir.ActivationFunctionType.Exp

    nch = s // CH
    nc = bacc.Bacc(None, target_bir_lowering=False)
    d_encT = nc.declare_dram_parameter("encT", [bl, H, s], b16, isOutput=False)
    d_weT = nc.declare_dram_parameter("weT", [H, H], b16, isOutput=False)
    d_whT = nc.declare_dram_parameter("whT", [H, H], f32, isOutput=False)
    d_hidT = nc.declare_dram_parameter("hidT", [H, bl], f32, isOutput=False)
    d_bT = nc.declare_dram_parameter("bT", [P, KB], f32, isOutput=False)
    d_vT = nc.declare_dram_parameter("vT", [P, KB, P], b16, isOutput=False)
    d_out = nc.declare_dram_parameter("out", [bl, s], f32, isOutput=True)

    with ExitStack() as ctx:
        tc = ctx.enter_context(tile.TileContext(nc))
        singles = ctx.enter_context(tc.tile_pool(name="singles", bufs=1))
        encp = ctx.enter_context(tc.tile_pool(name="encp", bufs=5))
        enp = ctx.enter_context(tc.tile_pool(name="energy", bufs=2 * KB))
        # ---- constants / weights ----
        # weT first on SP (its consumers are the very first main matmuls);
        # the h_proj/v weights go via the ACT engine's HWDGE port so SP can
        # move on to issuing the (many) enc DMAs.
        weT_sb, whT_sb, hidT_sb = [], [], []
        for hb in range(HB):
            w = singles.tile([P, H], b16, tag=f"weT{hb}")
            nc.sync.dma_start(out=w, in_=d_weT[hb * P:(hb + 1) * P, :])
            weT_sb.append(w)
        for hb in range(HB):
            wh = singles.tile([P, H], f32, tag=f"whT{hb}")
            nc.scalar.dma_start(out=wh, in_=d_whT[hb * P:(hb + 1) * P, :])
            whT_sb.append(wh)
            ht = singles.tile([P, bl], f32, tag=f"hidT{hb}")
            nc.scalar.dma_start(out=ht, in_=d_hidT[hb * P:(hb + 1) * P, :])
            hidT_sb.append(ht)
        bT_sb = singles.tile([P, KB], f32, tag="bT")
        nc.scalar.dma_start(out=bT_sb, in_=d_bT[:, :])
        vT_sb = singles.tile([P, KB, P], b16, tag="vT")
        nc.scalar.dma_start(out=vT_sb, in_=d_vT[:, :, :])
        vTf_sb = singles.tile([P, KB], f32, tag="vTf")
        nc.vector.tensor_copy(out=vTf_sb, in_=vT_sb[:, :, 0])
        ones_sb = singles.tile([P, P], b16, tag="ones")
        nc.vector.memset(ones_sb, 1.0)

        # ---- h_projT[k, (kb, b)] = Wh.T @ hidden.T + attn_b ----
        # hpsum pool is scoped: its PSUM bank is released back before the main
        # loop's pools get laid out... (bank budget: 6 epsum + 2 scpsum = 8)
        hproj_sb = singles.tile([P, KB * bl], f32, tag="hproj")
        with tc.tile_pool(name="hpsum", bufs=1, space="PSUM") as hpp:
            hps = hpp.tile([P, KB * bl], f32, tag="hp")
            for kb in range(KB):
                for hb in range(HB):
                    nc.tensor.matmul(
                        hps[:, kb * bl:(kb + 1) * bl],
                        lhsT=whT_sb[hb][:, kb * P:(kb + 1) * P],
                        rhs=hidT_sb[hb],
                        start=(hb == 0),
                        stop=(hb == HB - 1),
                    )
            for kb in range(KB):
                nc.vector.tensor_scalar_add(
                    out=hproj_sb[:, kb * bl:(kb + 1) * bl],
                    in0=hps[:, kb * bl:(kb + 1) * bl],
                    scalar1=bT_sb[:, kb:kb + 1],
                )

        # The matvec replicates each score row across all 128 partitions, so
        # every softmax tensor below is partition-replicated and every op runs
        # at base partition 0 on full tiles.
        scores_sb = singles.tile([P, s], f32, tag="scores")
        chmax_sb = singles.tile([P, bl * nch], f32, tag="chmax")   # max per chunk
        negmax_sb = singles.tile([P, bl * nch], f32, tag="negmax")
        dench_sb = singles.tile([P, bl * nch], f32, tag="dench")   # per-chunk exp sums
        mx = singles.tile([P, 1], f32, tag="mx")
        prob = singles.tile([P, s], f32, tag="prob")
        den = singles.tile([P, 1], f32, tag="den")
        inv = singles.tile([P, 1], f32, tag="inv")
        fb1 = singles.tile([P, nch], f32, tag="fb1")
        fb2 = singles.tile([P, nch], f32, tag="fb2")
        fb3 = singles.tile([P, nch], f32, tag="fb3")

        # ---- main loop: e_projT -> tanh -> v-dot ----
        prp = ctx.enter_context(tc.tile_pool(name="prod", bufs=4))
        ep = ctx.enter_context(tc.tile_pool(name="epsum", bufs=6, space="PSUM"))
        scp = ctx.enter_context(tc.tile_pool(name="scpsum", bufs=2, space="PSUM"))
        loop_cm = (
            tc.For_i(0, reps, 1, hint_engines=(mybir.EngineType.PE,))
            if reps > 1 else nullcontext()
        )
        ctx.enter_context(loop_cm)
        enc_tiles = [None] * HB
        for ibl in range(bl):
            for c in range(nch):
                sl = slice(c * CH, (c + 1) * CH)
                # enc is DMA'd in 4-chunk-wide (512KB) tiles: amortizes the
                # ~500ns HWDGE issue cost on SP while keeping prefetch deep.
                if c % 4 == 0:
                    w = min(4, nch - c)
                    sl2 = slice(c * CH, (c + w) * CH)
                    for hb in range(HB):
                        e = encp.tile([P, w * CH], b16, tag=f"enc{hb}")
                        nc.sync.dma_start(
                            out=e, in_=d_encT[ibl, hb * P:(hb + 1) * P, sl2]
                        )
                        enc_tiles[hb] = e
                enc_cs = [
                    enc_tiles[hb][:, (c % 4) * CH:(c % 4 + 1) * CH]
                    for hb in range(HB)
                ]
                en_tiles = []
                for kb in range(KB):
                    ps = ep.tile([P, CH], f32, tag="e")
                    for hb in range(HB):
                        nc.tensor.matmul(
                            ps,
                            lhsT=weT_sb[hb][:, kb * P:(kb + 1) * P],
                            rhs=enc_cs[hb],
                            start=(hb == 0),
                            stop=(hb == HB - 1),
                        )
                    en = enp.tile([P, CH], b16, tag="en")
                    nc.scalar.activation(
                        en, ps, Tanh,
                        bias=hproj_sb[:, kb * bl + ibl:kb * bl + ibl + 1],
                    )
                    en_tiles.append(en)
                # pre-combine the 4 k-blocks on DVE (x v[k], tree-sum);
                # the 128-partition contraction stays on PE as ONE ones-
                # stationary matmul (512 streamed columns instead of 2048)
                prods = []
                for kb in range(KB):
                    pr = prp.tile([P, CH], b16, tag=f"pr{kb}")
                    nc.vector.tensor_scalar_mul(
                        out=pr, in0=en_tiles[kb], scalar1=vTf_sb[:, kb:kb + 1]
                    )
                    prods.append(pr)
                a01 = prp.tile([P, CH], b16, tag="a01")
                nc.vector.tensor_add(a01, prods[0], prods[1])
                a23 = prp.tile([P, CH], b16, tag="a23")
                nc.vector.tensor_add(a23, prods[2], prods[3])
                asum = prp.tile([P, CH], b16, tag="asum")
                nc.vector.tensor_add(asum, a01, a23)
                sc = scp.tile([P, CH], f32, tag="sc")
                nc.tensor.matmul(sc, lhsT=ones_sb, rhs=asum)
                # flash-style softmax pass 1, straight off the PSUM tile and
                # overlapped with the next chunks' matmuls:
                # chmax = max_c, prob = exp(s - max_c), dench = sum(prob)
                col = slice(ibl * nch + c, ibl * nch + c + 1)
                nc.vector.reduce_max(
                    out=chmax_sb[:, col], in_=sc, axis=mybir.AxisListType.X
                )
                nc.vector.tensor_scalar_mul(
                    out=negmax_sb[:, col], in0=chmax_sb[:, col], scalar1=-1.0
                )
                nc.scalar.activation(
                    prob[:, sl], sc, Exp,
                    bias=negmax_sb[:, col], accum_out=dench_sb[:, col],
                )

            # ---- per-batch softmax epilogue (all rows are replicas) ----
            cs = slice(ibl * nch, (ibl + 1) * nch)
            nc.vector.reduce_max(
                out=mx, in_=chmax_sb[:, cs], axis=mybir.AxisListType.X
            )
            # f_c = exp(max_c - M);  den = sum_c dench_c * f_c;  g_c = f_c/den
            nc.vector.tensor_scalar_sub(out=fb1, in0=chmax_sb[:, cs], scalar1=mx)
            nc.scalar.activation(fb2, fb1, Exp)
            nc.vector.tensor_mul(fb3, fb2, dench_sb[:, cs])
            nc.vector.reduce_sum(out=den, in_=fb3, axis=mybir.AxisListType.X)
            nc.vector.reciprocal(inv, den)
            nc.vector.tensor_scalar_mul(out=fb2, in0=fb2, scalar1=inv)
            # out_c = prob_c * g_c  (writes all partition-replica rows); the
            # output DMA is split so its first half overlaps the second half's
            # rescale muls on the last batch
            for c in range(nch):
                sl = slice(c * CH, (c + 1) * CH)
                nc.vector.tensor_scalar_mul(
                    out=scores_sb[:, sl], in0=prob[:, sl],
                    scalar1=fb2[:, c:c + 1],
                )
                if c == nch // 2 - 1:
                    nc.sync.dma_start(
                        out=d_out[ibl, 0:s // 2], in_=scores_sb[0:1, 0:s // 2]
                    )
            nc.sync.dma_start(
                out=d_out[ibl, s // 2:], in_=scores_sb[0:1, s // 2:]
            )

    nc.compile()
    return nc


_CACHE = {}
LAST_RESULTS = None  # BassKernelResults of the most recent run (for profiling)


def _stage_host(hidden, encoder_outputs, attn_w, attn_b, v_w):
    hidden = np.asarray(hidden, dtype=np.float32)
    enc = np.asarray(encoder_outputs, dtype=np.float32)
    attn_w = np.asarray(attn_w, dtype=np.float32)
    attn_b = np.asarray(attn_b, dtype=np.float32)
    v_w = np.asarray(v_w, dtype=np.float32)

    weT = np.ascontiguousarray(attn_w[:, H:].T).astype(BF16)   # [h, k] bf16
    whT = np.ascontiguousarray(attn_w[:, :H].T)                # [h, k] f32
    bT = np.ascontiguousarray(attn_b.reshape(KB, P).T)         # [128, KB] f32
    vT = np.ascontiguousarray(
        np.broadcast_to(
            v_w[0].astype(BF16).reshape(KB, P).T[:, :, None], (P, KB, P)
        )
    )                                                          # [128, KB, 128] bf16
    encT = enc.transpose(0, 2, 1).astype(BF16)                 # [B, H, S] bf16

    in_maps = []
    for c in range(NCORES):
        lo = c * BL
        in_maps.append({
            "encT": encT[lo:lo + BL],
            "weT": weT,
            "whT": whT,
            "hidT": np.ascontiguousarray(hidden[lo:lo + BL].T),
            "bT": bT,
            "vT": vT,
        })
    return in_maps


def _get_runner(key="main", build=None):
    """Build (once per key) a persistently-jitted SPMD executor over 8 cores.

    Mirrors concourse.bass2jax.run_bass_via_pjrt's multi-core branch, but keeps
    the jitted callable alive so repeated invocations don't re-trace/compile.
    """
    cache_key = f"runner:{key}"
    if cache_key in _CACHE:
        return _CACHE[cache_key]

    import jax
    import concourse.mybir as mybir
    from concourse import bass2jax
    from jax.sharding import Mesh, PartitionSpec
    from jax.experimental.shard_map import shard_map

    bass2jax.install_neuronx_cc_hook()

    nc = build() if build is not None else build_nc()
    assert nc.dbg_addr is None

    partition_name = nc.partition_id_tensor.name if nc.partition_id_tensor else None
    in_names, out_names, out_avals, zero_shapes = [], [], [], []
    for alloc in nc.m.functions[0].allocations:
        if not isinstance(alloc, mybir.MemoryLocationSet):
            continue
        name = alloc.memorylocations[0].name
        if alloc.kind == "ExternalInput":
            if name != partition_name:
                in_names.append(name)
        elif alloc.kind == "ExternalOutput":
            shape = tuple(alloc.tensor_shape)
            dtype = mybir.dt.np(alloc.dtype)
            out_avals.append(jax.core.ShapedArray(shape, dtype))
            zero_shapes.append((shape, dtype))
            out_names.append(name)
    n_params = len(in_names)
    all_names = list(in_names) + list(out_names)
    if partition_name is not None:
        all_names.append(partition_name)

    def _body(*args):
        operands = list(args)
        if partition_name is not None:
            operands.append(bass2jax.partition_id_tensor())
        outs = bass2jax._bass_exec_p.bind(
            *operands,
            out_avals=tuple(out_avals),
            in_names=tuple(all_names),
            out_names=tuple(out_names),
            lowering_input_output_aliases=(),
            sim_require_finite=True,
            sim_require_nnan=True,
            nc=nc,
        )
        return tuple(outs)

    devices = jax.devices()[:NCORES]
    mesh = Mesh(np.asarray(devices), ("core",))
    n_outs = len(out_names)
    sharded = jax.jit(
        shard_map(
            _body,
            mesh=mesh,
            in_specs=(PartitionSpec("core"),) * (n_params + n_outs),
            out_specs=(PartitionSpec("core"),) * n_outs,
            check_rep=False,
        ),
        donate_argnums=tuple(range(n_params, n_params + n_outs)),
        keep_unused=True,
    )

    from jax.sharding import NamedSharding

    sharding = NamedSharding(mesh, PartitionSpec("core"))

    def prepare(in_maps):
        """Concatenate per-core inputs and place them on the devices."""
        concat_in = [
            np.concatenate([np.asarray(m[name]) for m in in_maps], axis=0)
            for name in in_names
        ]
        return [jax.device_put(a, sharding) for a in concat_in]

    def call(dev_in):
        concat_zeros = [
            np.zeros((NCORES * sh[0], *sh[1:]), dt) for (sh, dt) in zero_shapes
        ]
        out_arrs = sharded(*dev_in, *concat_zeros)
        return [
            {
                name: np.asarray(out_arrs[i]).reshape(NCORES, *out_avals[i].shape)[c]
                for i, name in enumerate(out_names)
            }
            for c in range(NCORES)
        ]

    def run(in_maps):
        return call(prepare(in_maps))

    run.prepare = prepare
    run.call = call
    _CACHE[cache_key] = run
    return run


def kernel(hidden, encoder_outputs, attn_w, attn_b, v_w):
    from concourse.bass_utils import run_bass_kernel_spmd

    if "nc" not in _CACHE:
        _CACHE["nc"] = build_nc()
    in_maps = _stage_host(hidden, encoder_outputs, attn_w, attn_b, v_w)
    res = run_bass_kernel_spmd(_CACHE["nc"], in_maps, list(range(NCORES)))
    out = np.concatenate([res.results[i]["out"] for i in range(NCORES)], axis=0)
    return np.ascontiguousarray(out.astype(np.float32))

